# revision 1
# baseline (speedup 1.0000x reference)
"""Trainium2 Bass kernel for nn_Loss_46883863003176.

loss = sum((predictions - targets)**2) / (2d+1) / batch_size
with predictions/targets of shape (4096, 2047, 2) float32.

Strategy (data-parallel over 8 NeuronCores):
  - Each core gets a contiguous batch shard of 512 rows, viewed flat as
    [128 partitions, 16376] f32 per tensor (33.5 MB HBM traffic per core).
  - Per tile of [128, F]: HWDGE DMA loads of pred/targ, DVE tensor_sub
    computes diff, ACT Square activation with accum_out produces the
    per-partition running sum of squares. Memory-bound; DVE and ACT each
    stay well under the ~93 us/core HBM roofline.
  - Each core writes a [128, NT] partial-sum tensor; host sums the 8
    partials in float64 and divides by (2d+1)*batch_size.
"""

import sys

if "/opt/trn_rl_repo" not in sys.path:
    sys.path.insert(0, "/opt/trn_rl_repo")

import numpy as np

B = 4096          # batch
S = 2047          # 2*d+1
C = 2             # coords
N_CORES = 8
ROWS = B // N_CORES          # 512 batch rows per core
PER_CORE = ROWS * S * C      # 2,096,128 elements
P = 128                      # SBUF partitions
FREE = PER_CORE // P         # 16376 elements per partition
NT = 4                       # tiles per core
F = FREE // NT               # 4094 elements per tile per partition

_CACHE = {}


def _build():
    import concourse.tile as tile
    from concourse import bacc, mybir

    nc = bacc.Bacc(
        "TRN2", debug=False, target_bir_lowering=False, num_devices=N_CORES
    )
    f32 = mybir.dt.float32
    p_ap = nc.dram_tensor("p", [P, FREE], f32, kind="ExternalInput").ap()
    t_ap = nc.dram_tensor("t", [P, FREE], f32, kind="ExternalInput").ap()
    acc_ap = nc.dram_tensor("acc", [P, NT], f32, kind="ExternalOutput").ap()

    with tile.TileContext(nc) as tc:
        with (
            tc.tile_pool(name="io", bufs=3) as io_pool,
            tc.tile_pool(name="work", bufs=2) as work,
            tc.tile_pool(name="accp", bufs=1) as accp,
        ):
            acc_sb = accp.tile([P, NT], f32)
            for j in range(NT):
                tp = io_pool.tile([P, F], f32, tag="p")
                nc.sync.dma_start(tp[:], p_ap[:, j * F : (j + 1) * F])
                tt = io_pool.tile([P, F], f32, tag="t")
                nc.sync.dma_start(tt[:], t_ap[:, j * F : (j + 1) * F])
                diff = work.tile([P, F], f32, tag="diff")
                nc.vector.tensor_sub(diff[:], tp[:], tt[:])
                sq = work.tile([P, F], f32, tag="sq")
                nc.scalar.activation(
                    sq[:],
                    diff[:],
                    mybir.ActivationFunctionType.Square,
                    accum_out=acc_sb[:, j : j + 1],
                )
            nc.sync.dma_start(acc_ap[:], acc_sb[:])
    nc.compile()
    return nc


def _get_nc():
    if "nc" not in _CACHE:
        _CACHE["nc"] = _build()
    return _CACHE["nc"]


def _shard(arr):
    # (B, S, C) contiguous -> 8 contiguous views of [128, FREE]
    return np.ascontiguousarray(arr).reshape(N_CORES, P, FREE)


def _run(in_maps, **kwargs):
    from concourse.bass_utils import run_bass_kernel_spmd

    return run_bass_kernel_spmd(_get_nc(), in_maps, list(range(N_CORES)), **kwargs)


def kernel(predictions, targets, d, batch_size, **_ignored):
    d_i = int(np.asarray(d))
    bs = int(np.asarray(batch_size))
    s_i = 2 * d_i + 1

    pred = np.asarray(predictions, dtype=np.float32)
    targ = np.asarray(targets, dtype=np.float32)

    if bs != B or s_i != S or pred.shape != (B, S, C):
        # Shape fell outside the compiled layout; numpy fallback keeps the
        # contract correct for any input.
        diff = (pred[:bs, :s_i, :C] - targ[:bs, :s_i, :C]).astype(np.float64)
        return np.float32((diff * diff).sum() / s_i / bs)

    pv = _shard(pred)
    tv = _shard(targ)
    in_maps = [{"p": pv[c], "t": tv[c]} for c in range(N_CORES)]
    res = _run(in_maps).results

    total = 0.0
    for r in res:
        total += float(r["acc"].astype(np.float64).sum())
    return np.float32(total / s_i / bs)


# revision 3
# speedup vs baseline: 1.0167x; 1.0167x over previous
"""Trainium2 Bass kernel for nn_Loss_46883863003176.

loss = sum((predictions - targets)**2) / (2d+1) / batch_size
with predictions/targets of shape (4096, 2047, 2) float32.

Strategy (data-parallel over 8 NeuronCores):
  - Each core gets a contiguous batch shard of 512 rows, viewed flat as
    [128 partitions, 16376] f32 per tensor (33.5 MB HBM traffic per core).
  - Per tile of [128, F]: HWDGE DMA loads of pred/targ, DVE tensor_sub
    computes diff, ACT Square activation with accum_out produces the
    per-partition running sum of squares. Memory-bound; DVE and ACT each
    stay well under the ~93 us/core HBM roofline.
  - Each core writes a [128, NT] partial-sum tensor; host sums the 8
    partials in float64 and divides by (2d+1)*batch_size.
"""

import sys

if "/opt/trn_rl_repo" not in sys.path:
    sys.path.insert(0, "/opt/trn_rl_repo")

import numpy as np

B = 4096          # batch
S = 2047          # 2*d+1
C = 2             # coords
N_CORES = 8
ROWS = B // N_CORES          # 512 batch rows per core
PER_CORE = ROWS * S * C      # 2,096,128 elements
P = 128                      # SBUF partitions
FREE = PER_CORE // P         # 16376 elements per partition
# Tapered tile sizes (elements per partition). Large tiles amortize DMA
# issue cost mid-stream; small trailing tiles shrink the compute tail that
# runs after the last DMA completes. Must sum to FREE.
TILE_SIZES = [2047, 4094, 4094, 4094, 1024, 1023]
assert sum(TILE_SIZES) == FREE
NT = len(TILE_SIZES)

_CACHE = {}


def _build():
    import concourse.tile as tile
    from concourse import bacc, mybir

    nc = bacc.Bacc(
        "TRN2", debug=False, target_bir_lowering=False, num_devices=N_CORES
    )
    f32 = mybir.dt.float32
    p_ap = nc.dram_tensor("p", [P, FREE], f32, kind="ExternalInput").ap()
    t_ap = nc.dram_tensor("t", [P, FREE], f32, kind="ExternalInput").ap()
    acc_ap = nc.dram_tensor("acc", [P, NT], f32, kind="ExternalOutput").ap()

    with tile.TileContext(nc) as tc:
        with (
            tc.tile_pool(name="io", bufs=3) as io_pool,
            tc.tile_pool(name="work", bufs=2) as work,
            tc.tile_pool(name="accp", bufs=1) as accp,
        ):
            acc_sb = accp.tile([P, NT], f32)
            fmax = max(TILE_SIZES)
            off = 0
            for j, f in enumerate(TILE_SIZES):
                tp = io_pool.tile([P, fmax], f32, tag="p")
                nc.sync.dma_start(tp[:, :f], p_ap[:, off : off + f])
                tt = io_pool.tile([P, fmax], f32, tag="t")
                nc.sync.dma_start(tt[:, :f], t_ap[:, off : off + f])
                diff = work.tile([P, fmax], f32, tag="diff")
                nc.vector.tensor_sub(diff[:, :f], tp[:, :f], tt[:, :f])
                sq = work.tile([P, fmax], f32, tag="sq")
                nc.scalar.activation(
                    sq[:, :f],
                    diff[:, :f],
                    mybir.ActivationFunctionType.Square,
                    accum_out=acc_sb[:, j : j + 1],
                )
                off += f
            nc.sync.dma_start(acc_ap[:], acc_sb[:])
    nc.compile()
    return nc


def _get_nc():
    if "nc" not in _CACHE:
        _CACHE["nc"] = _build()
    return _CACHE["nc"]


def _shard(arr):
    # (B, S, C) contiguous -> 8 contiguous views of [128, FREE]
    return np.ascontiguousarray(arr).reshape(N_CORES, P, FREE)


def _run(in_maps, **kwargs):
    from concourse.bass_utils import run_bass_kernel_spmd

    return run_bass_kernel_spmd(_get_nc(), in_maps, list(range(N_CORES)), **kwargs)


def kernel(predictions, targets, d, batch_size, **_ignored):
    d_i = int(np.asarray(d))
    bs = int(np.asarray(batch_size))
    s_i = 2 * d_i + 1

    pred = np.asarray(predictions, dtype=np.float32)
    targ = np.asarray(targets, dtype=np.float32)

    if bs != B or s_i != S or pred.shape != (B, S, C):
        # Shape fell outside the compiled layout; numpy fallback keeps the
        # contract correct for any input.
        diff = (pred[:bs, :s_i, :C] - targ[:bs, :s_i, :C]).astype(np.float64)
        return np.float32((diff * diff).sum() / s_i / bs)

    pv = _shard(pred)
    tv = _shard(targ)
    in_maps = [{"p": pv[c], "t": tv[c]} for c in range(N_CORES)]
    res = _run(in_maps).results

    total = 0.0
    for r in res:
        total += float(r["acc"].astype(np.float64).sum())
    return np.float32(total / s_i / bs)
